# revision 5
# baseline (speedup 1.0000x reference)
"""Trainium2 Bass kernel for nn_AdditiveAttention (B=16, LQ=1, LK=8192, D=H=1024).

scores[b, lk] = sum_h w_v[h] * tanh( (queries[b,0] @ W_q)[h] + (keys[b,lk] @ W_k)[h] )

Strategy:
  - Data-parallel over batch: 8 cores x 2 batches each. W_q/W_k/w_v replicated.
  - Host-side staging transposes each core's keys shard to [2, D, LK] so the
    contraction dim D lands on SBUF partitions (no on-chip transpose needed).
  - Matmuls run in fp16 (10-bit mantissa, fp32 PSUM accumulation; rel err
    ~5e-4 end to end). fp16 gets the fast weight load path so the per-matmul
    LDWEIGHTS (~97ns) hides under the 512-column moving stream (216ns).
    keysT is cast fp32->fp16 on VectorE right after DMA (hidden under PE).
  - Per 512-wide lk chunk: PE accumulates k-features [h_tile=128, 512] over
    the 8 d-chunks into a PSUM bank; ScalarE applies tanh with per-partition
    bias q[h] (PSUM -> SBUF fp16); PE contracts the 8 h_tiles with w_v into a
    [1, 512] score accumulator (issued 2 groups late so it never waits on
    ScalarE); VectorE evacuates it.
  - Startup: the first keys window is split into 512-wide slices so the first
    matmul group starts after ~6 MB of DMA, and the q projection is
    interleaved group-by-group into the first subchunk (W_q arrives as per-h
    column slices) to keep the PE dense while HAM warms up.
"""

import os
import sys

for _p in ("/opt/trn_rl_repo", "/root/.axon_site/_ro/trn_rl_repo"):
    if os.path.isdir(_p) and _p not in sys.path:
        sys.path.insert(0, _p)

import numpy as np
import concourse.bacc as bacc
import concourse.mybir as mybir
import concourse.tile as tile
from concourse.bass_utils import run_bass_kernel_spmd

B, LQ, LK, D, H = 16, 1, 8192, 1024, 1024
N_CORES = 8
NB = B // N_CORES      # batches per core
LKW = 2048             # steady-state lk window per DMA tile ([128, LKW] f32 = 1 MiB)
SUB = 512              # lk sub-chunk per PSUM bank
ND = D // 128
NH = H // 128
SCORE_LAG = 2          # issue score matmul for group h after main group h+SCORE_LAG

F16 = mybir.dt.float16
F32 = mybir.dt.float32
ACT_TANH = mybir.ActivationFunctionType.Tanh

_nc_cache = None
last_results = None    # BassKernelResults of the most recent run (for profiling)


def _gen_kernel():
    nc = bacc.Bacc("TRN2", target_bir_lowering=False, debug=False,
                   num_devices=N_CORES)
    keysT = nc.dram_tensor("keysT", [NB, D, LK], F32, kind="ExternalInput")
    queriesT = nc.dram_tensor("queriesT", [D, NB], F32, kind="ExternalInput")
    W_q = nc.dram_tensor("W_q", [D, H], F32, kind="ExternalInput")
    W_k = nc.dram_tensor("W_k", [D, H], F32, kind="ExternalInput")
    w_v = nc.dram_tensor("w_v", [H, 1], F32, kind="ExternalInput")
    scores = nc.dram_tensor("scores", [NB, LK], F32, kind="ExternalOutput")

    # (batch, lk_offset, lk_len); first window split small so compute starts early
    windows = [(0, 0, SUB), (0, SUB, SUB), (0, 2 * SUB, SUB), (0, 3 * SUB, SUB)]
    for w in range(1, LK // LKW):
        windows.append((0, w * LKW, LKW))
    for w in range(LK // LKW):
        windows.append((1, w * LKW, LKW))
    assert NB == 2

    with tile.TileContext(nc) as tc:
        with tc.tile_pool(name="wk", bufs=1) as wk_pool, \
             tc.tile_pool(name="const", bufs=1) as const_pool, \
             tc.tile_pool(name="qsetup", bufs=1) as qsetup_pool, \
             tc.tile_pool(name="keysf", bufs=5) as keysf_pool, \
             tc.tile_pool(name="keys", bufs=14) as keys_pool, \
             tc.tile_pool(name="feat", bufs=10) as feat_pool, \
             tc.tile_pool(name="outp", bufs=2) as out_pool, \
             tc.tile_pool(name="psf", bufs=4, space="PSUM") as psf_pool, \
             tc.tile_pool(name="psq", bufs=2, space="PSUM") as psq_pool, \
             tc.tile_pool(name="pss", bufs=2, space="PSUM") as pss_pool:

            def load_window(b, off, ln):
                tiles = []
                for d in range(ND):
                    tf = keysf_pool.tile([128, ln], F32, name="ktf", tag="ktf")
                    nc.sync.dma_start(
                        tf[:], keysT.ap()[b, d * 128:(d + 1) * 128, off:off + ln])
                    t = keys_pool.tile([128, ln], F16, name="kt", tag="kt")
                    nc.vector.tensor_copy(t[:], tf[:])
                    tiles.append(t)
                return tiles

            # --- DMA issue order on the sync (SP) HWDGE ring ---
            # 1) tiny: queriesT, w_v   2) W_k   3) first window slice
            # 4) W_q per-h column slices   5) remaining windows (in-loop)
            qsrc_f = qsetup_pool.tile([128, ND * NB], F32, name="qsrc_f")
            nc.sync.dma_start(
                qsrc_f[:].rearrange("p (c b) -> p c b", c=ND),
                queriesT.ap().rearrange("(c p) b -> p c b", p=128))
            qsrc = qsetup_pool.tile([128, ND * NB], F16, name="qsrc")
            nc.vector.tensor_copy(qsrc[:], qsrc_f[:])
            wv_sb = const_pool.tile([128, NH], F16, name="wv")
            wv_f = qsetup_pool.tile([128, NH], F32, name="wv_f32")
            nc.sync.dma_start(
                wv_f[:], w_v.ap().rearrange("(c p) o -> p (c o)", p=128))
            nc.vector.tensor_copy(wv_sb[:], wv_f[:])

            wk_sb = []
            for d in range(ND):
                tf = qsetup_pool.tile([128, H], F32, name="wk_f32", tag="wtmp",
                                      bufs=2)
                nc.sync.dma_start(tf[:], W_k.ap()[d * 128:(d + 1) * 128, :])
                t = wk_pool.tile([128, H], F16, name=f"wk{d}")
                nc.vector.tensor_copy(t[:], tf[:])
                wk_sb.append(t)

            pending = load_window(*windows[0])

            # W_q column slices (one per h group), cast to fp16
            wq_sb = []
            for h in range(NH):
                tf = qsetup_pool.tile([128, ND * 128], F32, name="wqf",
                                      tag="wqtmp", bufs=2)
                nc.sync.dma_start(
                    tf[:].rearrange("p (c x) -> p c x", c=ND),
                    W_q.ap().rearrange("(c p) hh -> p c hh", p=128)
                    [:, :, h * 128:(h + 1) * 128])
                t = qsetup_pool.tile([128, ND * 128], F16, name=f"wq{h}")
                nc.vector.tensor_copy(t[:], tf[:])
                wq_sb.append(t)

            qall = const_pool.tile([128, NH * NB], F32, name="qall")

            def emit_qproj(h):
                # qall[:, h*NB:(h+1)*NB] = sum_d W_q[d-chunk, h-cols].T @ queriesT
                ps_q = psq_pool.tile([128, NB], F32, name="ps_q")
                for d in range(ND):
                    nc.tensor.matmul(
                        ps_q[:], wq_sb[h][:, d * 128:(d + 1) * 128],
                        qsrc[:, d * NB:(d + 1) * NB],
                        start=(d == 0), stop=(d == ND - 1))
                nc.vector.tensor_copy(qall[:, h * NB:(h + 1) * NB], ps_q[:])

            for wi, (b, off, ln) in enumerate(windows):
                kt = pending
                if wi + 1 < len(windows):
                    pending = load_window(*windows[wi + 1])
                sc_sb = out_pool.tile([1, ln], F32, name="sc_sb", tag="sc")
                for sub in range(ln // SUB):
                    lo = sub * SUB
                    ps_s = pss_pool.tile([1, SUB], F32, name="ps_s")
                    score_q = []
                    for h in range(NH):
                        pf = psf_pool.tile([128, SUB], F32, name="pf")
                        for d in range(ND):
                            nc.tensor.matmul(
                                pf[:], wk_sb[d][:, h * 128:(h + 1) * 128],
                                kt[d][:, lo:lo + SUB],
                                start=(d == 0), stop=(d == ND - 1))
                        if wi == 0 and sub == 0:
                            # interleave q projection into the first subchunk:
                            # qall[h] is ready right before ACT(h) needs it
                            emit_qproj(h)
                        feat = feat_pool.tile([128, SUB], F16, name="feat")
                        nc.scalar.activation(
                            feat[:], pf[:], ACT_TANH,
                            bias=qall[:, h * NB + b:h * NB + b + 1])
                        score_q.append((h, feat))
                        if len(score_q) > SCORE_LAG:
                            hh, ff = score_q.pop(0)
                            nc.tensor.matmul(
                                ps_s[:], wv_sb[:, hh:hh + 1], ff[:],
                                start=(hh == 0), stop=(hh == NH - 1))
                    for hh, ff in score_q:
                        nc.tensor.matmul(
                            ps_s[:], wv_sb[:, hh:hh + 1], ff[:],
                            start=(hh == 0), stop=(hh == NH - 1))
                    nc.vector.tensor_copy(sc_sb[:, lo:lo + SUB], ps_s[:])
                nc.sync.dma_start(scores.ap()[b:b + 1, off:off + ln], sc_sb[:])
    nc.compile()
    return nc


def _get_nc():
    global _nc_cache
    if _nc_cache is None:
        _nc_cache = _gen_kernel()
    return _nc_cache


def kernel(queries, keys, W_q, W_k, w_v):
    global last_results
    queries = np.ascontiguousarray(np.asarray(queries, dtype=np.float32))
    keys = np.asarray(keys, dtype=np.float32)
    W_q = np.ascontiguousarray(np.asarray(W_q, dtype=np.float32))
    W_k = np.ascontiguousarray(np.asarray(W_k, dtype=np.float32))
    w_v = np.ascontiguousarray(np.asarray(w_v, dtype=np.float32))

    in_maps = []
    for c in range(N_CORES):
        b0 = c * NB
        keysT_c = np.ascontiguousarray(
            keys[b0:b0 + NB].transpose(0, 2, 1))          # [NB, D, LK]
        queriesT_c = np.ascontiguousarray(
            queries[b0:b0 + NB, 0, :].T)                  # [D, NB]
        in_maps.append({
            "keysT": keysT_c,
            "queriesT": queriesT_c,
            "W_q": W_q,
            "W_k": W_k,
            "w_v": w_v,
        })

    nc = _get_nc()
    res = run_bass_kernel_spmd(nc, in_maps, core_ids=list(range(N_CORES)))
    last_results = res
    return np.concatenate(
        [res.results[c]["scores"] for c in range(N_CORES)], axis=0)


if __name__ == "__main__":
    rng = np.random.default_rng(0)
    inputs = {
        "queries": rng.standard_normal((B, LQ, D), dtype=np.float32),
        "keys": rng.standard_normal((B, LK, D), dtype=np.float32),
        "W_q": (rng.standard_normal((D, H), dtype=np.float32) * 0.05),
        "W_k": (rng.standard_normal((D, H), dtype=np.float32) * 0.05),
        "w_v": (rng.standard_normal((H, 1), dtype=np.float32) * 0.05),
    }
    out = kernel(**inputs)
    print("out", out.shape, out.dtype, np.abs(out).mean())
